# revision 12
# baseline (speedup 1.0000x reference)
"""Distributed Bass kernel: causal multi-head attention with RoPE.

Full op:  x[2,2048,2048] -> attention(16 heads, RoPE, causal) @ wo.T
Sharding: core = b*4 + j  (b in {0,1} batch, j in {0..3} group-rank)
  - attention is head-sharded: core owns heads 4j..4j+3; QKV projections
    computed locally from x[b].T (replicated per group)
  - attention in "transposed scores" layout (scoresT[sk, sq]); softmax
    sums land broadcast across partitions via a 128-wide ones matmul
  - yT is AllGathered per seq-slice (one gather per slice, issued as the
    slice's last head finishes) so the collective chain runs back to back
    under the attention compute; the output projection for slice s is
    emitted after attention, ordered so the PE reaches it only after its
    gather has landed.
  - attention is software-pipelined: scores of chunk-pair p+1 issue
    before the sums/AV of pair p, so the PE stays busy during ACT exp
Compute dtype bf16 (f32 accumulation in PSUM); inputs converted on host.
"""

import math
import os
import sys

for _p in ("/opt/trn_rl_repo",):
    if _p not in sys.path:
        sys.path.insert(0, _p)

import ml_dtypes
import numpy as np

import concourse.bass as bass  # noqa: F401
import concourse.mybir as mybir
import concourse.tile as tile
from concourse import bacc
from concourse.bass_utils import run_bass_kernel_spmd

BF16 = mybir.dt.bfloat16
F32 = mybir.dt.float32
NPBF16 = ml_dtypes.bfloat16

B, S, D = 2, 2048, 2048
H, HD = 16, 128
BASE = 10000
NCORES = 8
GROUPS = [[0, 1, 2, 3], [4, 5, 6, 7]]
HPC = 4            # heads per core
DPC = HPC * HD     # 512 hidden dims per core
KC = D // 128      # 16 contraction chunks
NS = S // 512      # 4 seq slices of 512
SCALE = 1.0 / math.sqrt(HD)
NEG = -30000.0

_CACHE = {}

LAST_EXEC_NS = None
LAST_TRACE = None


def _install_ntff_hook():
    """The image's antenv lacks axon_hooks; bass_utils hard-imports it when
    trace=True. Register the boot module's ctypes hook under that name."""
    try:
        import antenv.axon_hooks  # noqa: F401
        return True
    except ImportError:
        pass
    try:
        import types

        import antenv
        from trn_agent_boot.trn_boot import _ntff_profile_via_ctypes

        mod = types.ModuleType("antenv.axon_hooks")
        _hook = [None]
        mod.set_axon_ntff_profile_hook = lambda h: _hook.__setitem__(0, h)
        mod.get_axon_ntff_profile_hook = lambda: _hook[0]
        sys.modules["antenv.axon_hooks"] = mod
        antenv.axon_hooks = mod
        mod.set_axon_ntff_profile_hook(
            _ntff_profile_via_ctypes("/opt/axon/libaxon_pjrt.so")
        )
        return True
    except Exception:
        return False


def _build():
    nc = bacc.Bacc(None, target_bir_lowering=False, num_devices=NCORES)

    xT = nc.declare_dram_parameter("xT", [D, S], BF16, isOutput=False)
    wqT = nc.declare_dram_parameter("wqT", [D, DPC], BF16, isOutput=False)
    wkT = nc.declare_dram_parameter("wkT", [D, DPC], BF16, isOutput=False)
    wvT = nc.declare_dram_parameter("wvT", [D, DPC], BF16, isOutput=False)
    woT = nc.declare_dram_parameter("woT", [D, DPC], BF16, isOutput=False)
    cosE = nc.declare_dram_parameter("cosE", [HD, S], BF16, isOutput=False)
    sinE = nc.declare_dram_parameter("sinE", [HD, S], BF16, isOutput=False)
    pswap = nc.declare_dram_parameter("pswap", [128, 128], BF16, isOutput=False)
    btri = nc.declare_dram_parameter("btri", [128, 128], BF16, isOutput=False)
    ident = nc.declare_dram_parameter("ident", [128, 128], BF16, isOutput=False)
    ones2 = nc.declare_dram_parameter("ones2", [128, 128], BF16, isOutput=False)
    out = nc.declare_dram_parameter("out", [S, DPC], F32, isOutput=True)

    with tile.TileContext(nc) as tc:
        with (
            tc.tile_pool(name="consts", bufs=1) as cpool,
            tc.tile_pool(name="qkv", bufs=1) as qkvp,
            tc.tile_pool(name="dram", bufs=1, space="DRAM") as dpool,
            tc.tile_pool(name="ytout", bufs=4) as ytp,
        ):
            # small aux consts on the gpsimd DMA queue (x/w use sync+scalar)
            p_t = cpool.tile([128, 128], BF16, tag="pswap", name="pswap")
            nc.gpsimd.dma_start(out=p_t[:], in_=pswap[:, :])
            btri_t = cpool.tile([128, 128], BF16, tag="btri", name="btri")
            nc.gpsimd.dma_start(out=btri_t[:], in_=btri[:, :])
            id_t = cpool.tile([128, 128], BF16, tag="ident", name="ident")
            nc.gpsimd.dma_start(out=id_t[:], in_=ident[:, :])
            ones_t = cpool.tile([128, 128], BF16, tag="ones2", name="ones2")
            nc.gpsimd.dma_start(out=ones_t[:], in_=ones2[:, :])

            # tiny dummy collective issued first: absorbs the CC engine's
            # ~80us startup cost while projections run.
            warm_in = dpool.tile([128, 2], BF16, tag="warm_in", name="warm_in")
            warm_out = dpool.tile([512, 2], BF16, tag="warm_out", name="warm_out")
            nc.gpsimd.dma_start(out=warm_in[:], in_=ones2[:, 0:2])
            nc.gpsimd.collective_compute(
                "AllGather",
                mybir.AluOpType.bypass,
                replica_groups=GROUPS,
                ins=[warm_in[:].opt()],
                outs=[warm_out[:].opt()],
            )

            # per-slice AllGather staging: binS[s] rows h*128 = my head-h yT
            # for slice s; boutS[s] rows r*512+h*128 = global head 4r+h.
            binS = [dpool.tile([DPC, 512], BF16, tag=f"binS{s}", name=f"binS{s}")
                    for s in range(NS)]
            boutS = [dpool.tile([4 * DPC, 512], BF16, tag=f"boutS{s}", name=f"boutS{s}")
                     for s in range(NS)]

            qT = [qkvp.tile([HD, S], BF16, tag=f"q{h}", name=f"q{h}") for h in range(HPC)]
            kT = [qkvp.tile([HD, S], BF16, tag=f"k{h}", name=f"k{h}") for h in range(HPC)]
            vv = [qkvp.tile([128, DPC], BF16, tag=f"v{m}", name=f"v{m}") for m in range(KC)]

            # ---------------- phase 1: projections + RoPE ----------------
            with (
                tc.tile_pool(name="xt", bufs=1) as xtp,
                tc.tile_pool(name="wgt", bufs=1) as wp,
                tc.tile_pool(name="rtmp", bufs=2) as rtmp,
            ):
                # DMA order matched to compute order: wv[k] then x[k] pieces
                # (V consumes them first), wq/wk afterwards.  Issue spread
                # over the three DMA-capable queues; x tiles split in 4 so
                # each ~22GB/s DMA engine moves a smaller piece and the
                # chunk lands sooner.
                qs = [nc.sync, nc.scalar, nc.gpsimd]
                xt = []
                w_t = {"q": [], "k": [], "v": []}
                for k in range(KC):
                    t = wp.tile([128, DPC], BF16, tag=f"wv{k}", name=f"wv{k}")
                    for pc in range(2):
                        qs[(k + pc) % 3].dma_start(
                            out=t[:, pc * 256:(pc + 1) * 256],
                            in_=wvT[k * 128:(k + 1) * 128, pc * 256:(pc + 1) * 256],
                        )
                    w_t["v"].append(t)
                    t = xtp.tile([128, S], BF16, tag=f"x{k}", name=f"x{k}")
                    for pc in range(4):
                        qs[(k + pc + 2) % 3].dma_start(
                            out=t[:, pc * 512:(pc + 1) * 512],
                            in_=xT[k * 128:(k + 1) * 128, pc * 512:(pc + 1) * 512],
                        )
                    xt.append(t)
                # cos/sin needed first at the rope (~QK start); issue after
                # x/wv so they don't delay the V-projection stream.
                cos_t = cpool.tile([HD, S], BF16, tag="cos", name="cos")
                sin_t = cpool.tile([HD, S], BF16, tag="sin", name="sin")
                for pc in range(4):
                    qs[pc % 3].dma_start(
                        out=cos_t[:, pc * 512:(pc + 1) * 512],
                        in_=cosE[:, pc * 512:(pc + 1) * 512],
                    )
                    qs[(pc + 1) % 3].dma_start(
                        out=sin_t[:, pc * 512:(pc + 1) * 512],
                        in_=sinE[:, pc * 512:(pc + 1) * 512],
                    )
                for k in range(KC):
                    t = wp.tile([128, DPC], BF16, tag=f"wk{k}", name=f"wk{k}")
                    qs[k % 3].dma_start(out=t[:], in_=wkT[k * 128:(k + 1) * 128, :])
                    w_t["k"].append(t)
                    t = wp.tile([128, DPC], BF16, tag=f"wq{k}", name=f"wq{k}")
                    qs[(k + 1) % 3].dma_start(out=t[:], in_=wqT[k * 128:(k + 1) * 128, :])
                    w_t["q"].append(t)

                # v projection, k-outer round-robin over 8 psum banks so the
                # PE consumes each x/wv chunk right as its DMA lands.
                with tc.tile_pool(name="ps8", bufs=1, space="PSUM") as ps8:
                    for rnd in range(2):
                        pv = [ps8.tile([128, DPC], F32, tag=f"pv{mi}", name=f"pv{mi}")
                              for mi in range(8)]
                        for k in range(KC):
                            for mi in range(8):
                                m = rnd * 8 + mi
                                nc.tensor.matmul(
                                    pv[mi][:], xt[k][:, m * 128:(m + 1) * 128],
                                    w_t["v"][k][:],
                                    start=(k == 0), stop=(k == KC - 1),
                                    skip_group_check=True,
                                )
                        for mi in range(8):
                            nc.vector.tensor_copy(vv[rnd * 8 + mi][:], pv[mi][:])

                # q/k projections (transposed layout) + RoPE.  The rope for
                # group N is emitted after group N+1's matmuls so the P-swap
                # matmul never stalls the PE stream on the DVE psum copy.
                with (
                    tc.tile_pool(name="ps1", bufs=2, space="PSUM") as ps1,
                    tc.tile_pool(name="psw", bufs=2, space="PSUM") as psw,
                ):
                    def rope_tail(pq, dst, h, n):
                        raw = rtmp.tile([128, 512], BF16, tag="raw", name="raw")
                        nc.vector.tensor_copy(raw[:], pq[:])
                        sw = psw.tile([128, 512], F32, tag="sw", name="sw")
                        nc.tensor.matmul(sw[:], p_t[:], raw[:], start=True, stop=True)
                        t1 = rtmp.tile([128, 512], BF16, tag="t1", name="t1")
                        nc.vector.tensor_tensor(
                            t1[:], raw[:], cos_t[:, n * 512:(n + 1) * 512],
                            mybir.AluOpType.mult,
                        )
                        t2 = rtmp.tile([128, 512], BF16, tag="t2", name="t2")
                        nc.vector.tensor_tensor(
                            t2[:], sw[:], sin_t[:, n * 512:(n + 1) * 512],
                            mybir.AluOpType.mult,
                        )
                        nc.vector.tensor_tensor(
                            dst[h][:, n * 512:(n + 1) * 512], t1[:], t2[:],
                            mybir.AluOpType.add,
                        )

                    pending = None
                    for h in range(HPC):
                        for nm, dst in (("k", kT), ("q", qT)):
                            for n in range(NS):
                                pq = ps1.tile([128, 512], F32, tag="pq", name="pq")
                                for k in range(KC):
                                    nc.tensor.matmul(
                                        pq[:],
                                        w_t[nm][k][:, h * 128:(h + 1) * 128],
                                        xt[k][:, n * 512:(n + 1) * 512],
                                        start=(k == 0), stop=(k == KC - 1),
                                    )
                                if pending is not None:
                                    rope_tail(*pending)
                                pending = (pq, dst, h, n)
                    if pending is not None:
                        rope_tail(*pending)

            # wo chunks: issued after the phase-1 loads so they don't steal
            # HBM bandwidth from x/w; they arrive long before phase 3.
            wo_t = []
            for g in range(KC):
                t = cpool.tile([128, DPC], BF16, tag=f"wo{g}", name=f"wo{g}")
                [nc.sync, nc.scalar][g % 2].dma_start(
                    out=t[:], in_=woT[g * 128:(g + 1) * 128, :]
                )
                wo_t.append(t)

            # ---------------- phase 2+3: attention, alltoalls, out-proj ----
            with (
                tc.tile_pool(name="pssc", bufs=2, space="PSUM") as pssc,
                tc.tile_pool(name="psyt", bufs=2, space="PSUM") as psyt,
                tc.tile_pool(name="psmb", bufs=1, space="PSUM") as psmb,
                tc.tile_pool(name="pso", bufs=1, space="PSUM") as pso,
                tc.tile_pool(name="probs", bufs=8) as prp,
                tc.tile_pool(name="fin", bufs=2) as finp,
                tc.tile_pool(name="yts", bufs=1) as ytsp,
                tc.tile_pool(name="ost", bufs=2) as ostp,
            ):
                ytk = {}

                def emit_sc(s, h, p):
                    sq0 = s * 512
                    psc = pssc.tile([128, 1024], F32, tag="psc", name="psc")
                    dlts = []
                    for half in range(2):
                        kk = 2 * p + half
                        off = half * 512
                        diag = kk >= s * 4
                        dlt = (kk - s * 4) * 128 if diag else 0
                        dlts.append(dlt)
                        nc.tensor.matmul(
                            psc[:, off + dlt:off + 512],
                            kT[h][:, kk * 128:(kk + 1) * 128],
                            qT[h][:, sq0 + dlt:sq0 + 512],
                            start=True, stop=not diag,
                        )
                        if diag:
                            nc.tensor.matmul(
                                psc[:, off + dlt:off + dlt + 128],
                                id_t[:], btri_t[:],
                                start=False, stop=True,
                            )
                    return psc, dlts

                def emit_exp(psc, dlts):
                    pb = prp.tile([128, 1024], BF16, tag="pb", name="pb")
                    nc.scalar.activation(
                        pb[:, dlts[0]:1024], psc[:, dlts[0]:1024],
                        mybir.ActivationFunctionType.Exp, scale=SCALE,
                    )
                    return pb

                def emit_con(s, h, p, pb, dlts, pyt, psm):
                    nchunks = (s + 1) * 4
                    for half in range(2):
                        kk = 2 * p + half
                        off = half * 512
                        dlt = dlts[half]
                        nc.tensor.matmul(
                            psm[:, dlt:512], ones_t[:],
                            pb[:, off + dlt:off + 512],
                            start=(kk == 0), stop=(kk == nchunks - 1),
                        )
                        nc.tensor.matmul(
                            pyt[:, dlt:512],
                            vv[kk][:, h * 128:(h + 1) * 128],
                            pb[:, off + dlt:off + 512],
                            start=(kk == 0), stop=(kk == nchunks - 1),
                        )

                def emit_epilogue(s, h, pyt, psm):
                    rcp = finp.tile([128, 512], F32, tag="rcp", name="rcp")
                    nc.vector.reciprocal_approx_fast(out=rcp[:], in_=psm[:])
                    yt = ytp.tile([128, 512], BF16, tag="yt", name="yt")
                    nc.vector.tensor_tensor(
                        yt[:], pyt[:], rcp[:], mybir.AluOpType.mult,
                    )
                    nc.gpsimd.dma_start(
                        out=binS[s][h * HD:(h + 1) * HD, :], in_=yt[:],
                    )

                def emit_gather(s):
                    nc.gpsimd.collective_compute(
                        "AllGather",
                        mybir.AluOpType.bypass,
                        replica_groups=GROUPS,
                        ins=[binS[s][:].opt()],
                        outs=[boutS[s][:].opt()],
                    )

                def load_ytk(s):
                    # staged on the gpsimd SW-DGE queues, emitted after the
                    # last gather trigger: a gather-gated load at a HW-queue
                    # head would otherwise block later phase-3 output DMAs
                    # (head-of-line).
                    for hh in range(HPC):
                        for r in range(4):
                            g = 4 * r + hh
                            t = ytsp.tile([128, 512], BF16, tag=f"ytk{s}_{g}",
                                          name=f"ytk{s}_{g}")
                            nc.gpsimd.dma_start(
                                out=t[:],
                                in_=boutS[s][r * DPC + hh * HD:
                                             r * DPC + (hh + 1) * HD, :],
                            )
                            ytk[(s, g)] = t

                # flattened (slc, head, chunk-pair) stream, software-pipelined
                # one pair deep: scores+exp of pair i issue before sums/AV of
                # pair i-1, so the PE never waits on the ACT exp.
                items = [(s, h, p)
                         for s in range(NS) for h in range(HPC)
                         for p in range((s + 1) * 2)]
                pend = None
                upsum = {}
                for it in items:
                    s, h, p = it
                    if p == 0:
                        upsum[(s, h)] = (
                            psyt.tile([128, 512], F32, tag="pyt", name="pyt"),
                            psmb.tile([128, 512], F32, tag="psm", name="psm"),
                        )
                    psc, dlts = emit_sc(s, h, p)
                    pb = emit_exp(psc, dlts)
                    if pend is not None:
                        ps_, ph_, pp_, ppb, pdlts = pend
                        pyt, psm = upsum[(ps_, ph_)]
                        emit_con(ps_, ph_, pp_, ppb, pdlts, pyt, psm)
                        if pp_ == (ps_ + 1) * 2 - 1:   # last pair of unit
                            emit_epilogue(ps_, ph_, pyt, psm)
                            if ph_ == 3:
                                emit_gather(ps_)
                    pend = (s, h, p, pb, dlts)
                ps_, ph_, pp_, ppb, pdlts = pend
                pyt, psm = upsum[(ps_, ph_)]
                emit_con(ps_, ph_, pp_, ppb, pdlts, pyt, psm)
                emit_epilogue(ps_, ph_, pyt, psm)
                emit_gather(3)
                for s3 in range(NS):
                    load_ytk(s3)

                # phase 3, after attention: out rows 4s+j; the per-slice
                # blocks are ordered (tile_wait_until) so the PE reaches
                # block s only after its gather has landed.
                gs = [4 * r + hh for hh in range(HPC) for r in range(4)]
                for s3 in range(NS):
                    with tc.tile_wait_until(100.0 + s3):
                        for j in range(4):
                            m = 4 * s3 + j
                            po = pso.tile([128, 512], F32, tag="po", name="po") \
                                if j % 2 == 0 else \
                                psyt.tile([128, 512], F32, tag="pyt", name="po")
                            for i, g in enumerate(gs):
                                nc.tensor.matmul(
                                    po[:],
                                    ytk[(s3, g)][:, j * 128:(j + 1) * 128],
                                    wo_t[g][:],
                                    start=(i == 0), stop=(i == 15),
                                )
                            ot = ostp.tile([128, 512], F32, tag="ot", name="ot")
                            nc.scalar.copy(ot[:], po[:])
                            nc.sync.dma_start(
                                out=out[m * 128:(m + 1) * 128, 0:256],
                                in_=ot[:, 0:256],
                            )
                            nc.scalar.dma_start(
                                out=out[m * 128:(m + 1) * 128, 256:512],
                                in_=ot[:, 256:512],
                            )

    nc.finalize()
    return nc


def _host_consts():
    theta = 1.0 / (BASE ** (np.arange(0, HD, 2, dtype=np.float64)[: HD // 2] / HD))
    idx = np.arange(S, dtype=np.float64)[:, None] * theta[None, :]  # [S, 64]
    cos = np.cos(idx).astype(np.float32)
    sin = np.sin(idx).astype(np.float32)
    cosE = np.repeat(cos.T, 2, axis=0)          # [128, S]
    sinE = np.repeat(sin.T, 2, axis=0)
    sinE[0::2, :] *= -1.0                        # even rows: -sin
    P = np.zeros((128, 128), np.float32)
    P[np.arange(128), np.arange(128) ^ 1] = 1.0
    btri = np.where(
        np.arange(128)[:, None] > np.arange(128)[None, :], NEG, 0.0
    ).astype(np.float32)
    ident = np.eye(128, dtype=np.float32)
    ones2 = np.ones((128, 128), np.float32)
    return {
        "cosE": cosE.astype(NPBF16),
        "sinE": sinE.astype(NPBF16),
        "pswap": P.astype(NPBF16),
        "btri": btri.astype(NPBF16),
        "ident": ident.astype(NPBF16),
        "ones2": ones2.astype(NPBF16),
    }


def kernel(x, mask, wq, wk, wv, wo):
    global LAST_EXEC_NS, LAST_TRACE
    x = np.asarray(x, dtype=np.float32)
    wq = np.asarray(wq, dtype=np.float32)
    wk = np.asarray(wk, dtype=np.float32)
    wv = np.asarray(wv, dtype=np.float32)
    wo = np.asarray(wo, dtype=np.float32)

    consts = _host_consts()
    in_maps = []
    for core in range(NCORES):
        b, j = core // 4, core % 4
        sl = slice(j * DPC, (j + 1) * DPC)
        m = {
            "xT": np.ascontiguousarray(x[b].T).astype(NPBF16),
            "wqT": np.ascontiguousarray(wq[sl, :].T).astype(NPBF16),
            "wkT": np.ascontiguousarray(wk[sl, :].T).astype(NPBF16),
            "wvT": np.ascontiguousarray(wv[sl, :].T).astype(NPBF16),
            "woT": np.ascontiguousarray(wo[sl, :].T).astype(NPBF16),
        }
        m.update(consts)
        in_maps.append(m)

    if "nc" not in _CACHE:
        _CACHE["nc"] = _build()
    nc = _CACHE["nc"]

    trace = os.environ.get("KERNEL_TRACE", "0") == "1"
    if trace:
        trace = _install_ntff_hook()
    res = run_bass_kernel_spmd(
        nc, in_maps, core_ids=list(range(NCORES)), trace=trace,
    )
    LAST_EXEC_NS = getattr(res, "exec_time_ns", None)
    LAST_TRACE = getattr(res, "instructions_and_trace", None)

    out = np.empty((B, S, D), np.float32)
    for core in range(NCORES):
        b, j = core // 4, core % 4
        out[b, :, j * DPC:(j + 1) * DPC] = np.asarray(
            res.results[core]["out"], dtype=np.float32
        )
    return out


# revision 13
# speedup vs baseline: 1.0029x; 1.0029x over previous
"""Distributed Bass kernel: causal multi-head attention with RoPE.

Full op:  x[2,2048,2048] -> attention(16 heads, RoPE, causal) @ wo.T
Sharding: core = b*4 + j  (b in {0,1} batch, j in {0..3} group-rank)
  - attention is head-sharded: core owns heads 4j..4j+3; QKV projections
    computed locally from x[b].T (replicated per group)
  - attention in "transposed scores" layout (scoresT[sk, sq]); softmax
    sums land broadcast across partitions via a 128-wide ones matmul
  - yT is AllGathered per seq-slice (one gather per slice, issued as the
    slice's last head finishes) so the collective chain runs back to back
    under the attention compute; the output projection for slice s is
    emitted after attention, ordered so the PE reaches it only after its
    gather has landed.
  - attention is software-pipelined: scores of chunk-pair p+1 issue
    before the sums/AV of pair p, so the PE stays busy during ACT exp
Compute dtype bf16 (f32 accumulation in PSUM); inputs converted on host.
"""

import math
import os
import sys

for _p in ("/opt/trn_rl_repo",):
    if _p not in sys.path:
        sys.path.insert(0, _p)

import ml_dtypes
import numpy as np

import concourse.bass as bass  # noqa: F401
import concourse.mybir as mybir
import concourse.tile as tile
from concourse import bacc
from concourse.bass_utils import run_bass_kernel_spmd

BF16 = mybir.dt.bfloat16
F32 = mybir.dt.float32
NPBF16 = ml_dtypes.bfloat16

B, S, D = 2, 2048, 2048
H, HD = 16, 128
BASE = 10000
NCORES = 8
GROUPS = [[0, 1, 2, 3], [4, 5, 6, 7]]
HPC = 4            # heads per core
DPC = HPC * HD     # 512 hidden dims per core
KC = D // 128      # 16 contraction chunks
NS = S // 512      # 4 seq slices of 512
SCALE = 1.0 / math.sqrt(HD)
NEG = -30000.0

_CACHE = {}

LAST_EXEC_NS = None
LAST_TRACE = None


def _install_ntff_hook():
    """The image's antenv lacks axon_hooks; bass_utils hard-imports it when
    trace=True. Register the boot module's ctypes hook under that name."""
    try:
        import antenv.axon_hooks  # noqa: F401
        return True
    except ImportError:
        pass
    try:
        import types

        import antenv
        from trn_agent_boot.trn_boot import _ntff_profile_via_ctypes

        mod = types.ModuleType("antenv.axon_hooks")
        _hook = [None]
        mod.set_axon_ntff_profile_hook = lambda h: _hook.__setitem__(0, h)
        mod.get_axon_ntff_profile_hook = lambda: _hook[0]
        sys.modules["antenv.axon_hooks"] = mod
        antenv.axon_hooks = mod
        mod.set_axon_ntff_profile_hook(
            _ntff_profile_via_ctypes("/opt/axon/libaxon_pjrt.so")
        )
        return True
    except Exception:
        return False


def _build():
    nc = bacc.Bacc(None, target_bir_lowering=False, num_devices=NCORES)

    xT = nc.declare_dram_parameter("xT", [D, S], BF16, isOutput=False)
    wqT = nc.declare_dram_parameter("wqT", [D, DPC], BF16, isOutput=False)
    wkT = nc.declare_dram_parameter("wkT", [D, DPC], BF16, isOutput=False)
    wvT = nc.declare_dram_parameter("wvT", [D, DPC], BF16, isOutput=False)
    woT = nc.declare_dram_parameter("woT", [D, DPC], BF16, isOutput=False)
    cosE = nc.declare_dram_parameter("cosE", [HD, S], BF16, isOutput=False)
    sinE = nc.declare_dram_parameter("sinE", [HD, S], BF16, isOutput=False)
    pswap = nc.declare_dram_parameter("pswap", [128, 128], BF16, isOutput=False)
    btri = nc.declare_dram_parameter("btri", [128, 128], BF16, isOutput=False)
    ident = nc.declare_dram_parameter("ident", [128, 128], BF16, isOutput=False)
    ones2 = nc.declare_dram_parameter("ones2", [128, 128], BF16, isOutput=False)
    out = nc.declare_dram_parameter("out", [S, DPC], F32, isOutput=True)

    with tile.TileContext(nc) as tc:
        with (
            tc.tile_pool(name="consts", bufs=1) as cpool,
            tc.tile_pool(name="qkv", bufs=1) as qkvp,
            tc.tile_pool(name="dram", bufs=1, space="DRAM") as dpool,
            tc.tile_pool(name="ytout", bufs=4) as ytp,
        ):
            # small aux consts on the gpsimd DMA queue (x/w use sync+scalar)
            cos_t = cpool.tile([HD, S], BF16, tag="cos", name="cos")
            nc.gpsimd.dma_start(out=cos_t[:], in_=cosE[:, :])
            sin_t = cpool.tile([HD, S], BF16, tag="sin", name="sin")
            nc.gpsimd.dma_start(out=sin_t[:], in_=sinE[:, :])
            p_t = cpool.tile([128, 128], BF16, tag="pswap", name="pswap")
            nc.gpsimd.dma_start(out=p_t[:], in_=pswap[:, :])
            btri_t = cpool.tile([128, 128], BF16, tag="btri", name="btri")
            nc.gpsimd.dma_start(out=btri_t[:], in_=btri[:, :])
            id_t = cpool.tile([128, 128], BF16, tag="ident", name="ident")
            nc.gpsimd.dma_start(out=id_t[:], in_=ident[:, :])
            ones_t = cpool.tile([128, 128], BF16, tag="ones2", name="ones2")
            nc.gpsimd.dma_start(out=ones_t[:], in_=ones2[:, :])

            # tiny dummy collective issued first: absorbs the CC engine's
            # ~80us startup cost while projections run.
            warm_in = dpool.tile([128, 2], BF16, tag="warm_in", name="warm_in")
            warm_out = dpool.tile([512, 2], BF16, tag="warm_out", name="warm_out")
            nc.gpsimd.dma_start(out=warm_in[:], in_=ones2[:, 0:2])
            nc.gpsimd.collective_compute(
                "AllGather",
                mybir.AluOpType.bypass,
                replica_groups=GROUPS,
                ins=[warm_in[:].opt()],
                outs=[warm_out[:].opt()],
            )

            # per-slice AllGather staging: binS[s] rows h*128 = my head-h yT
            # for slice s; boutS[s] rows r*512+h*128 = global head 4r+h.
            binS = [dpool.tile([DPC, 512], BF16, tag=f"binS{s}", name=f"binS{s}")
                    for s in range(NS)]
            boutS = [dpool.tile([4 * DPC, 512], BF16, tag=f"boutS{s}", name=f"boutS{s}")
                     for s in range(NS)]

            qT = [qkvp.tile([HD, S], BF16, tag=f"q{h}", name=f"q{h}") for h in range(HPC)]
            kT = [qkvp.tile([HD, S], BF16, tag=f"k{h}", name=f"k{h}") for h in range(HPC)]
            vv = [qkvp.tile([128, DPC], BF16, tag=f"v{m}", name=f"v{m}") for m in range(KC)]

            # ---------------- phase 1: projections + RoPE ----------------
            with (
                tc.tile_pool(name="xt", bufs=1) as xtp,
                tc.tile_pool(name="wgt", bufs=1) as wp,
                tc.tile_pool(name="rtmp", bufs=2) as rtmp,
            ):
                # DMA order matched to compute order: wv[k] then x[k] pieces
                # (V consumes them first), wq/wk afterwards.  Issue spread
                # over the three DMA-capable queues; x tiles split in 4 so
                # each ~22GB/s DMA engine moves a smaller piece and the
                # chunk lands sooner.
                qs = [nc.sync, nc.scalar, nc.gpsimd]
                xt = []
                w_t = {"q": [], "k": [], "v": []}
                for k in range(KC):
                    t = wp.tile([128, DPC], BF16, tag=f"wv{k}", name=f"wv{k}")
                    qs[k % 3].dma_start(out=t[:], in_=wvT[k * 128:(k + 1) * 128, :])
                    w_t["v"].append(t)
                    t = xtp.tile([128, S], BF16, tag=f"x{k}", name=f"x{k}")
                    for pc in range(4):
                        qs[(k + pc + 1) % 3].dma_start(
                            out=t[:, pc * 512:(pc + 1) * 512],
                            in_=xT[k * 128:(k + 1) * 128, pc * 512:(pc + 1) * 512],
                        )
                    xt.append(t)
                for k in range(KC):
                    t = wp.tile([128, DPC], BF16, tag=f"wq{k}", name=f"wq{k}")
                    qs[k % 3].dma_start(out=t[:], in_=wqT[k * 128:(k + 1) * 128, :])
                    w_t["q"].append(t)
                    t = wp.tile([128, DPC], BF16, tag=f"wk{k}", name=f"wk{k}")
                    qs[(k + 1) % 3].dma_start(out=t[:], in_=wkT[k * 128:(k + 1) * 128, :])
                    w_t["k"].append(t)

                # v projection, k-outer round-robin over 8 psum banks so the
                # PE consumes each x/wv chunk right as its DMA lands.
                with tc.tile_pool(name="ps8", bufs=1, space="PSUM") as ps8:
                    for rnd in range(2):
                        pv = [ps8.tile([128, DPC], F32, tag=f"pv{mi}", name=f"pv{mi}")
                              for mi in range(8)]
                        for k in range(KC):
                            for mi in range(8):
                                m = rnd * 8 + mi
                                nc.tensor.matmul(
                                    pv[mi][:], xt[k][:, m * 128:(m + 1) * 128],
                                    w_t["v"][k][:],
                                    start=(k == 0), stop=(k == KC - 1),
                                    skip_group_check=True,
                                )
                        for mi in range(8):
                            nc.vector.tensor_copy(vv[rnd * 8 + mi][:], pv[mi][:])

                # q/k projections (transposed layout) + RoPE.  The rope for
                # group N is emitted after group N+1's matmuls so the P-swap
                # matmul never stalls the PE stream on the DVE psum copy.
                with (
                    tc.tile_pool(name="ps1", bufs=2, space="PSUM") as ps1,
                    tc.tile_pool(name="psw", bufs=2, space="PSUM") as psw,
                ):
                    def rope_tail(pq, dst, h, n):
                        raw = rtmp.tile([128, 512], BF16, tag="raw", name="raw")
                        nc.vector.tensor_copy(raw[:], pq[:])
                        sw = psw.tile([128, 512], F32, tag="sw", name="sw")
                        nc.tensor.matmul(sw[:], p_t[:], raw[:], start=True, stop=True)
                        t1 = rtmp.tile([128, 512], BF16, tag="t1", name="t1")
                        nc.vector.tensor_tensor(
                            t1[:], raw[:], cos_t[:, n * 512:(n + 1) * 512],
                            mybir.AluOpType.mult,
                        )
                        t2 = rtmp.tile([128, 512], BF16, tag="t2", name="t2")
                        nc.vector.tensor_tensor(
                            t2[:], sw[:], sin_t[:, n * 512:(n + 1) * 512],
                            mybir.AluOpType.mult,
                        )
                        nc.vector.tensor_tensor(
                            dst[h][:, n * 512:(n + 1) * 512], t1[:], t2[:],
                            mybir.AluOpType.add,
                        )

                    pending = None
                    for h in range(HPC):
                        for nm, dst in (("k", kT), ("q", qT)):
                            for n in range(NS):
                                pq = ps1.tile([128, 512], F32, tag="pq", name="pq")
                                for k in range(KC):
                                    nc.tensor.matmul(
                                        pq[:],
                                        w_t[nm][k][:, h * 128:(h + 1) * 128],
                                        xt[k][:, n * 512:(n + 1) * 512],
                                        start=(k == 0), stop=(k == KC - 1),
                                    )
                                if pending is not None:
                                    rope_tail(*pending)
                                pending = (pq, dst, h, n)
                    if pending is not None:
                        rope_tail(*pending)

            # wo chunks: issued after the phase-1 loads so they don't steal
            # HBM bandwidth from x/w; they arrive long before phase 3.
            wo_t = []
            for g in range(KC):
                t = cpool.tile([128, DPC], BF16, tag=f"wo{g}", name=f"wo{g}")
                [nc.sync, nc.scalar][g % 2].dma_start(
                    out=t[:], in_=woT[g * 128:(g + 1) * 128, :]
                )
                wo_t.append(t)

            # ---------------- phase 2+3: attention, alltoalls, out-proj ----
            with (
                tc.tile_pool(name="pssc", bufs=2, space="PSUM") as pssc,
                tc.tile_pool(name="psyt", bufs=2, space="PSUM") as psyt,
                tc.tile_pool(name="psmb", bufs=1, space="PSUM") as psmb,
                tc.tile_pool(name="pso", bufs=1, space="PSUM") as pso,
                tc.tile_pool(name="probs", bufs=8) as prp,
                tc.tile_pool(name="fin", bufs=2) as finp,
                tc.tile_pool(name="yts", bufs=1) as ytsp,
                tc.tile_pool(name="ost", bufs=2) as ostp,
            ):
                ytk = {}

                def emit_sc(s, h, p):
                    sq0 = s * 512
                    psc = pssc.tile([128, 1024], F32, tag="psc", name="psc")
                    dlts = []
                    for half in range(2):
                        kk = 2 * p + half
                        off = half * 512
                        diag = kk >= s * 4
                        dlt = (kk - s * 4) * 128 if diag else 0
                        dlts.append(dlt)
                        nc.tensor.matmul(
                            psc[:, off + dlt:off + 512],
                            kT[h][:, kk * 128:(kk + 1) * 128],
                            qT[h][:, sq0 + dlt:sq0 + 512],
                            start=True, stop=not diag,
                        )
                        if diag:
                            nc.tensor.matmul(
                                psc[:, off + dlt:off + dlt + 128],
                                id_t[:], btri_t[:],
                                start=False, stop=True,
                            )
                    return psc, dlts

                def emit_exp(psc, dlts):
                    pb = prp.tile([128, 1024], BF16, tag="pb", name="pb")
                    nc.scalar.activation(
                        pb[:, dlts[0]:1024], psc[:, dlts[0]:1024],
                        mybir.ActivationFunctionType.Exp, scale=SCALE,
                    )
                    return pb

                def emit_con(s, h, p, pb, dlts, pyt, psm):
                    nchunks = (s + 1) * 4
                    for half in range(2):
                        kk = 2 * p + half
                        off = half * 512
                        dlt = dlts[half]
                        nc.tensor.matmul(
                            psm[:, dlt:512], ones_t[:],
                            pb[:, off + dlt:off + 512],
                            start=(kk == 0), stop=(kk == nchunks - 1),
                        )
                        nc.tensor.matmul(
                            pyt[:, dlt:512],
                            vv[kk][:, h * 128:(h + 1) * 128],
                            pb[:, off + dlt:off + 512],
                            start=(kk == 0), stop=(kk == nchunks - 1),
                        )

                def emit_epilogue(s, h, pyt, psm):
                    rcp = finp.tile([128, 512], F32, tag="rcp", name="rcp")
                    nc.vector.reciprocal_approx_fast(out=rcp[:], in_=psm[:])
                    yt = ytp.tile([128, 512], BF16, tag="yt", name="yt")
                    nc.vector.tensor_tensor(
                        yt[:], pyt[:], rcp[:], mybir.AluOpType.mult,
                    )
                    nc.gpsimd.dma_start(
                        out=binS[s][h * HD:(h + 1) * HD, :], in_=yt[:],
                    )

                def emit_gather(s):
                    nc.gpsimd.collective_compute(
                        "AllGather",
                        mybir.AluOpType.bypass,
                        replica_groups=GROUPS,
                        ins=[binS[s][:].opt()],
                        outs=[boutS[s][:].opt()],
                    )

                def load_ytk(s):
                    # staged on the gpsimd SW-DGE queues, emitted after the
                    # last gather trigger: a gather-gated load at a HW-queue
                    # head would otherwise block later phase-3 output DMAs
                    # (head-of-line).
                    for hh in range(HPC):
                        for r in range(4):
                            g = 4 * r + hh
                            t = ytsp.tile([128, 512], BF16, tag=f"ytk{s}_{g}",
                                          name=f"ytk{s}_{g}")
                            nc.gpsimd.dma_start(
                                out=t[:],
                                in_=boutS[s][r * DPC + hh * HD:
                                             r * DPC + (hh + 1) * HD, :],
                            )
                            ytk[(s, g)] = t

                # flattened (slc, head, chunk-pair) stream, software-pipelined
                # one pair deep: scores+exp of pair i issue before sums/AV of
                # pair i-1, so the PE never waits on the ACT exp.
                items = [(s, h, p)
                         for s in range(NS) for h in range(HPC)
                         for p in range((s + 1) * 2)]
                pend = None
                upsum = {}
                for it in items:
                    s, h, p = it
                    if p == 0:
                        upsum[(s, h)] = (
                            psyt.tile([128, 512], F32, tag="pyt", name="pyt"),
                            psmb.tile([128, 512], F32, tag="psm", name="psm"),
                        )
                    psc, dlts = emit_sc(s, h, p)
                    pb = emit_exp(psc, dlts)
                    if pend is not None:
                        ps_, ph_, pp_, ppb, pdlts = pend
                        pyt, psm = upsum[(ps_, ph_)]
                        emit_con(ps_, ph_, pp_, ppb, pdlts, pyt, psm)
                        if pp_ == (ps_ + 1) * 2 - 1:   # last pair of unit
                            emit_epilogue(ps_, ph_, pyt, psm)
                            if ph_ == 3:
                                emit_gather(ps_)
                    pend = (s, h, p, pb, dlts)
                ps_, ph_, pp_, ppb, pdlts = pend
                pyt, psm = upsum[(ps_, ph_)]
                emit_con(ps_, ph_, pp_, ppb, pdlts, pyt, psm)
                emit_epilogue(ps_, ph_, pyt, psm)
                emit_gather(3)
                for s3 in range(NS):
                    load_ytk(s3)

                # phase 3, after attention: out rows 4s+j; the per-slice
                # blocks are ordered (tile_wait_until) so the PE reaches
                # block s only after its gather has landed.
                gs = [4 * r + hh for hh in range(HPC) for r in range(4)]
                for s3 in range(NS):
                    with tc.tile_wait_until(100.0 + s3):
                        for j in range(4):
                            m = 4 * s3 + j
                            po = pso.tile([128, 512], F32, tag="po", name="po") \
                                if j % 2 == 0 else \
                                psyt.tile([128, 512], F32, tag="pyt", name="po")
                            for i, g in enumerate(gs):
                                nc.tensor.matmul(
                                    po[:],
                                    ytk[(s3, g)][:, j * 128:(j + 1) * 128],
                                    wo_t[g][:],
                                    start=(i == 0), stop=(i == 15),
                                )
                            ot = ostp.tile([128, 512], F32, tag="ot", name="ot")
                            nc.scalar.copy(ot[:], po[:])
                            nc.sync.dma_start(
                                out=out[m * 128:(m + 1) * 128, 0:256],
                                in_=ot[:, 0:256],
                            )
                            nc.scalar.dma_start(
                                out=out[m * 128:(m + 1) * 128, 256:512],
                                in_=ot[:, 256:512],
                            )

    nc.finalize()
    return nc


def _host_consts():
    theta = 1.0 / (BASE ** (np.arange(0, HD, 2, dtype=np.float64)[: HD // 2] / HD))
    idx = np.arange(S, dtype=np.float64)[:, None] * theta[None, :]  # [S, 64]
    cos = np.cos(idx).astype(np.float32)
    sin = np.sin(idx).astype(np.float32)
    cosE = np.repeat(cos.T, 2, axis=0)          # [128, S]
    sinE = np.repeat(sin.T, 2, axis=0)
    sinE[0::2, :] *= -1.0                        # even rows: -sin
    P = np.zeros((128, 128), np.float32)
    P[np.arange(128), np.arange(128) ^ 1] = 1.0
    btri = np.where(
        np.arange(128)[:, None] > np.arange(128)[None, :], NEG, 0.0
    ).astype(np.float32)
    ident = np.eye(128, dtype=np.float32)
    ones2 = np.ones((128, 128), np.float32)
    return {
        "cosE": cosE.astype(NPBF16),
        "sinE": sinE.astype(NPBF16),
        "pswap": P.astype(NPBF16),
        "btri": btri.astype(NPBF16),
        "ident": ident.astype(NPBF16),
        "ones2": ones2.astype(NPBF16),
    }


def kernel(x, mask, wq, wk, wv, wo):
    global LAST_EXEC_NS, LAST_TRACE
    x = np.asarray(x, dtype=np.float32)
    wq = np.asarray(wq, dtype=np.float32)
    wk = np.asarray(wk, dtype=np.float32)
    wv = np.asarray(wv, dtype=np.float32)
    wo = np.asarray(wo, dtype=np.float32)

    consts = _host_consts()
    in_maps = []
    for core in range(NCORES):
        b, j = core // 4, core % 4
        sl = slice(j * DPC, (j + 1) * DPC)
        m = {
            "xT": np.ascontiguousarray(x[b].T).astype(NPBF16),
            "wqT": np.ascontiguousarray(wq[sl, :].T).astype(NPBF16),
            "wkT": np.ascontiguousarray(wk[sl, :].T).astype(NPBF16),
            "wvT": np.ascontiguousarray(wv[sl, :].T).astype(NPBF16),
            "woT": np.ascontiguousarray(wo[sl, :].T).astype(NPBF16),
        }
        m.update(consts)
        in_maps.append(m)

    if "nc" not in _CACHE:
        _CACHE["nc"] = _build()
    nc = _CACHE["nc"]

    trace = os.environ.get("KERNEL_TRACE", "0") == "1"
    if trace:
        trace = _install_ntff_hook()
    res = run_bass_kernel_spmd(
        nc, in_maps, core_ids=list(range(NCORES)), trace=trace,
    )
    LAST_EXEC_NS = getattr(res, "exec_time_ns", None)
    LAST_TRACE = getattr(res, "instructions_and_trace", None)

    out = np.empty((B, S, D), np.float32)
    for core in range(NCORES):
        b, j = core // 4, core % 4
        out[b, :, j * DPC:(j + 1) * DPC] = np.asarray(
            res.results[core]["out"], dtype=np.float32
        )
    return out


# revision 14
# speedup vs baseline: 1.0435x; 1.0404x over previous
"""Distributed Bass kernel: causal multi-head attention with RoPE.

Full op:  x[2,2048,2048] -> attention(16 heads, RoPE, causal) @ wo.T
Sharding: core = b*4 + j  (b in {0,1} batch, j in {0..3} group-rank)
  - attention is head-sharded: core owns heads 4j..4j+3; QKV projections
    computed locally from x[b].T (replicated per group)
  - attention in "transposed scores" layout (scoresT[sk, sq]); softmax
    sums land broadcast across partitions via a 128-wide ones matmul
  - yT is AllGathered per seq-slice (one gather per slice, issued as the
    slice's last head finishes) so the collective chain runs back to back
    under the attention compute; the output projection for slice s is
    emitted after attention, ordered so the PE reaches it only after its
    gather has landed.
  - attention is software-pipelined: scores of chunk-pair p+1 issue
    before the sums/AV of pair p, so the PE stays busy during ACT exp
Compute dtype bf16 (f32 accumulation in PSUM); inputs converted on host.
"""

import math
import os
import sys

for _p in ("/opt/trn_rl_repo",):
    if _p not in sys.path:
        sys.path.insert(0, _p)

import ml_dtypes
import numpy as np

import concourse.bass as bass  # noqa: F401
import concourse.mybir as mybir
import concourse.tile as tile
from concourse import bacc
from concourse.bass_utils import run_bass_kernel_spmd

BF16 = mybir.dt.bfloat16
F32 = mybir.dt.float32
NPBF16 = ml_dtypes.bfloat16

B, S, D = 2, 2048, 2048
H, HD = 16, 128
BASE = 10000
NCORES = 8
GROUPS = [[0, 1, 2, 3], [4, 5, 6, 7]]
HPC = 4            # heads per core
DPC = HPC * HD     # 512 hidden dims per core
KC = D // 128      # 16 contraction chunks
NS = S // 512      # 4 seq slices of 512
SCALE = 1.0 / math.sqrt(HD)
NEG = -30000.0

_CACHE = {}

LAST_EXEC_NS = None
LAST_TRACE = None


def _install_ntff_hook():
    """The image's antenv lacks axon_hooks; bass_utils hard-imports it when
    trace=True. Register the boot module's ctypes hook under that name."""
    try:
        import antenv.axon_hooks  # noqa: F401
        return True
    except ImportError:
        pass
    try:
        import types

        import antenv
        from trn_agent_boot.trn_boot import _ntff_profile_via_ctypes

        mod = types.ModuleType("antenv.axon_hooks")
        _hook = [None]
        mod.set_axon_ntff_profile_hook = lambda h: _hook.__setitem__(0, h)
        mod.get_axon_ntff_profile_hook = lambda: _hook[0]
        sys.modules["antenv.axon_hooks"] = mod
        antenv.axon_hooks = mod
        mod.set_axon_ntff_profile_hook(
            _ntff_profile_via_ctypes("/opt/axon/libaxon_pjrt.so")
        )
        return True
    except Exception:
        return False


def _build():
    nc = bacc.Bacc(None, target_bir_lowering=False, num_devices=NCORES)

    xT = nc.declare_dram_parameter("xT", [D, S], BF16, isOutput=False)
    wqT = nc.declare_dram_parameter("wqT", [D, DPC], BF16, isOutput=False)
    wkT = nc.declare_dram_parameter("wkT", [D, DPC], BF16, isOutput=False)
    wvT = nc.declare_dram_parameter("wvT", [D, DPC], BF16, isOutput=False)
    woT = nc.declare_dram_parameter("woT", [D, DPC], BF16, isOutput=False)
    cosE = nc.declare_dram_parameter("cosE", [HD, S], BF16, isOutput=False)
    sinE = nc.declare_dram_parameter("sinE", [HD, S], BF16, isOutput=False)
    pswap = nc.declare_dram_parameter("pswap", [128, 128], BF16, isOutput=False)
    btri = nc.declare_dram_parameter("btri", [128, 128], BF16, isOutput=False)
    ident = nc.declare_dram_parameter("ident", [128, 128], BF16, isOutput=False)
    ones2 = nc.declare_dram_parameter("ones2", [128, 128], BF16, isOutput=False)
    out = nc.declare_dram_parameter("out", [S, DPC], F32, isOutput=True)

    with tile.TileContext(nc) as tc:
        with (
            tc.tile_pool(name="consts", bufs=1) as cpool,
            tc.tile_pool(name="qkv", bufs=1) as qkvp,
            tc.tile_pool(name="dram", bufs=1, space="DRAM") as dpool,
            tc.tile_pool(name="ytout", bufs=4) as ytp,
        ):
            # small aux consts on the gpsimd DMA queue (x/w use sync+scalar)
            cos_t = cpool.tile([HD, S], BF16, tag="cos", name="cos")
            sin_t = cpool.tile([HD, S], BF16, tag="sin", name="sin")
            p_t = cpool.tile([128, 128], BF16, tag="pswap", name="pswap")
            nc.gpsimd.dma_start(out=p_t[:], in_=pswap[:, :])
            btri_t = cpool.tile([128, 128], BF16, tag="btri", name="btri")
            nc.gpsimd.dma_start(out=btri_t[:], in_=btri[:, :])
            id_t = cpool.tile([128, 128], BF16, tag="ident", name="ident")
            nc.gpsimd.dma_start(out=id_t[:], in_=ident[:, :])
            ones_t = cpool.tile([128, 128], BF16, tag="ones2", name="ones2")
            nc.gpsimd.dma_start(out=ones_t[:], in_=ones2[:, :])

            # tiny dummy collective issued first: absorbs the CC engine's
            # ~80us startup cost while projections run.
            warm_in = dpool.tile([128, 2], BF16, tag="warm_in", name="warm_in")
            warm_out = dpool.tile([512, 2], BF16, tag="warm_out", name="warm_out")
            nc.gpsimd.dma_start(out=warm_in[:], in_=ones2[:, 0:2])
            nc.gpsimd.collective_compute(
                "AllGather",
                mybir.AluOpType.bypass,
                replica_groups=GROUPS,
                ins=[warm_in[:].opt()],
                outs=[warm_out[:].opt()],
            )

            # per-slice AllGather staging: binS[s] rows h*128 = my head-h yT
            # for slice s; boutS[s] rows r*512+h*128 = global head 4r+h.
            binS = [dpool.tile([DPC, 512], BF16, tag=f"binS{s}", name=f"binS{s}")
                    for s in range(NS)]
            boutS = [dpool.tile([4 * DPC, 512], BF16, tag=f"boutS{s}", name=f"boutS{s}")
                     for s in range(NS)]

            qT = [qkvp.tile([HD, S], BF16, tag=f"q{h}", name=f"q{h}") for h in range(HPC)]
            kT = [qkvp.tile([HD, S], BF16, tag=f"k{h}", name=f"k{h}") for h in range(HPC)]
            vv = [qkvp.tile([128, DPC], BF16, tag=f"v{m}", name=f"v{m}") for m in range(KC)]

            # ---------------- phase 1: projections + RoPE ----------------
            with (
                tc.tile_pool(name="xt", bufs=1) as xtp,
                tc.tile_pool(name="wgt", bufs=1) as wp,
                tc.tile_pool(name="rtmp", bufs=2) as rtmp,
            ):
                # DMA order matched to compute order: wv[k] then x[k] pieces
                # (V consumes them first), wq/wk afterwards.  Issue spread
                # over the three DMA-capable queues; x tiles split in 4 so
                # each ~22GB/s DMA engine moves a smaller piece and the
                # chunk lands sooner.
                qs = [nc.sync, nc.scalar, nc.gpsimd]
                xt = []
                w_t = {"q": [], "k": [], "v": []}
                for k in range(KC):
                    t = wp.tile([128, DPC], BF16, tag=f"wv{k}", name=f"wv{k}")
                    qs[k % 3].dma_start(out=t[:], in_=wvT[k * 128:(k + 1) * 128, :])
                    w_t["v"].append(t)
                    t = xtp.tile([128, S], BF16, tag=f"x{k}", name=f"x{k}")
                    for pc in range(4):
                        qs[(k + pc + 1) % 3].dma_start(
                            out=t[:, pc * 512:(pc + 1) * 512],
                            in_=xT[k * 128:(k + 1) * 128, pc * 512:(pc + 1) * 512],
                        )
                    xt.append(t)
                # cos/sin (1MB) issued after x/wv so they never delay the
                # V stream; needed first at the rope, well after they land.
                for pc in range(4):
                    nc.sync.dma_start(
                        out=cos_t[:, pc * 512:(pc + 1) * 512],
                        in_=cosE[:, pc * 512:(pc + 1) * 512],
                    )
                    nc.scalar.dma_start(
                        out=sin_t[:, pc * 512:(pc + 1) * 512],
                        in_=sinE[:, pc * 512:(pc + 1) * 512],
                    )
                for k in range(KC):
                    t = wp.tile([128, DPC], BF16, tag=f"wq{k}", name=f"wq{k}")
                    qs[k % 3].dma_start(out=t[:], in_=wqT[k * 128:(k + 1) * 128, :])
                    w_t["q"].append(t)
                    t = wp.tile([128, DPC], BF16, tag=f"wk{k}", name=f"wk{k}")
                    qs[(k + 1) % 3].dma_start(out=t[:], in_=wkT[k * 128:(k + 1) * 128, :])
                    w_t["k"].append(t)

                # v projection, k-outer round-robin over 8 psum banks so the
                # PE consumes each x/wv chunk right as its DMA lands.
                with tc.tile_pool(name="ps8", bufs=1, space="PSUM") as ps8:
                    for rnd in range(2):
                        pv = [ps8.tile([128, DPC], F32, tag=f"pv{mi}", name=f"pv{mi}")
                              for mi in range(8)]
                        for k in range(KC):
                            for mi in range(8):
                                m = rnd * 8 + mi
                                nc.tensor.matmul(
                                    pv[mi][:], xt[k][:, m * 128:(m + 1) * 128],
                                    w_t["v"][k][:],
                                    start=(k == 0), stop=(k == KC - 1),
                                    skip_group_check=True,
                                )
                        for mi in range(8):
                            nc.vector.tensor_copy(vv[rnd * 8 + mi][:], pv[mi][:])

                # q/k projections (transposed layout) + RoPE.  The rope for
                # group N is emitted after group N+1's matmuls so the P-swap
                # matmul never stalls the PE stream on the DVE psum copy.
                with (
                    tc.tile_pool(name="ps1", bufs=2, space="PSUM") as ps1,
                    tc.tile_pool(name="psw", bufs=2, space="PSUM") as psw,
                ):
                    def rope_tail(pq, dst, h, n):
                        raw = rtmp.tile([128, 512], BF16, tag="raw", name="raw")
                        nc.vector.tensor_copy(raw[:], pq[:])
                        sw = psw.tile([128, 512], F32, tag="sw", name="sw")
                        nc.tensor.matmul(sw[:], p_t[:], raw[:], start=True, stop=True)
                        t1 = rtmp.tile([128, 512], BF16, tag="t1", name="t1")
                        nc.vector.tensor_tensor(
                            t1[:], raw[:], cos_t[:, n * 512:(n + 1) * 512],
                            mybir.AluOpType.mult,
                        )
                        t2 = rtmp.tile([128, 512], BF16, tag="t2", name="t2")
                        nc.vector.tensor_tensor(
                            t2[:], sw[:], sin_t[:, n * 512:(n + 1) * 512],
                            mybir.AluOpType.mult,
                        )
                        nc.vector.tensor_tensor(
                            dst[h][:, n * 512:(n + 1) * 512], t1[:], t2[:],
                            mybir.AluOpType.add,
                        )

                    pending = None
                    for h in range(HPC):
                        for nm, dst in (("k", kT), ("q", qT)):
                            for n in range(NS):
                                pq = ps1.tile([128, 512], F32, tag="pq", name="pq")
                                for k in range(KC):
                                    nc.tensor.matmul(
                                        pq[:],
                                        w_t[nm][k][:, h * 128:(h + 1) * 128],
                                        xt[k][:, n * 512:(n + 1) * 512],
                                        start=(k == 0), stop=(k == KC - 1),
                                    )
                                if pending is not None:
                                    rope_tail(*pending)
                                pending = (pq, dst, h, n)
                    if pending is not None:
                        rope_tail(*pending)

            # wo chunks: issued after the phase-1 loads so they don't steal
            # HBM bandwidth from x/w; they arrive long before phase 3.
            wo_t = []
            for g in range(KC):
                t = cpool.tile([128, DPC], BF16, tag=f"wo{g}", name=f"wo{g}")
                [nc.sync, nc.scalar][g % 2].dma_start(
                    out=t[:], in_=woT[g * 128:(g + 1) * 128, :]
                )
                wo_t.append(t)

            # ---------------- phase 2+3: attention, alltoalls, out-proj ----
            with (
                tc.tile_pool(name="pssc", bufs=2, space="PSUM") as pssc,
                tc.tile_pool(name="psyt", bufs=2, space="PSUM") as psyt,
                tc.tile_pool(name="psmb", bufs=1, space="PSUM") as psmb,
                tc.tile_pool(name="pso", bufs=1, space="PSUM") as pso,
                tc.tile_pool(name="probs", bufs=8) as prp,
                tc.tile_pool(name="fin", bufs=2) as finp,
                tc.tile_pool(name="yts", bufs=1) as ytsp,
                tc.tile_pool(name="ost", bufs=2) as ostp,
            ):
                ytk = {}

                def emit_sc(s, h, p):
                    sq0 = s * 512
                    psc = pssc.tile([128, 1024], F32, tag="psc", name="psc")
                    dlts = []
                    for half in range(2):
                        kk = 2 * p + half
                        off = half * 512
                        diag = kk >= s * 4
                        dlt = (kk - s * 4) * 128 if diag else 0
                        dlts.append(dlt)
                        nc.tensor.matmul(
                            psc[:, off + dlt:off + 512],
                            kT[h][:, kk * 128:(kk + 1) * 128],
                            qT[h][:, sq0 + dlt:sq0 + 512],
                            start=True, stop=not diag,
                        )
                        if diag:
                            nc.tensor.matmul(
                                psc[:, off + dlt:off + dlt + 128],
                                id_t[:], btri_t[:],
                                start=False, stop=True,
                            )
                    return psc, dlts

                def emit_exp(psc, dlts):
                    pb = prp.tile([128, 1024], BF16, tag="pb", name="pb")
                    nc.scalar.activation(
                        pb[:, dlts[0]:1024], psc[:, dlts[0]:1024],
                        mybir.ActivationFunctionType.Exp, scale=SCALE,
                    )
                    return pb

                def emit_con(s, h, p, pb, dlts, pyt, psm):
                    nchunks = (s + 1) * 4
                    for half in range(2):
                        kk = 2 * p + half
                        off = half * 512
                        dlt = dlts[half]
                        nc.tensor.matmul(
                            psm[:, dlt:512], ones_t[:],
                            pb[:, off + dlt:off + 512],
                            start=(kk == 0), stop=(kk == nchunks - 1),
                        )
                        nc.tensor.matmul(
                            pyt[:, dlt:512],
                            vv[kk][:, h * 128:(h + 1) * 128],
                            pb[:, off + dlt:off + 512],
                            start=(kk == 0), stop=(kk == nchunks - 1),
                        )

                def emit_epilogue(s, h, pyt, psm):
                    rcp = finp.tile([128, 512], F32, tag="rcp", name="rcp")
                    nc.vector.reciprocal_approx_fast(out=rcp[:], in_=psm[:])
                    yt = ytp.tile([128, 512], BF16, tag="yt", name="yt")
                    nc.vector.tensor_tensor(
                        yt[:], pyt[:], rcp[:], mybir.AluOpType.mult,
                    )
                    nc.gpsimd.dma_start(
                        out=binS[s][h * HD:(h + 1) * HD, :], in_=yt[:],
                    )

                def emit_gather(s):
                    nc.gpsimd.collective_compute(
                        "AllGather",
                        mybir.AluOpType.bypass,
                        replica_groups=GROUPS,
                        ins=[binS[s][:].opt()],
                        outs=[boutS[s][:].opt()],
                    )

                def load_ytk(s):
                    # staged on the gpsimd SW-DGE queues, emitted after the
                    # last gather trigger: a gather-gated load at a HW-queue
                    # head would otherwise block later phase-3 output DMAs
                    # (head-of-line).
                    for hh in range(HPC):
                        for r in range(4):
                            g = 4 * r + hh
                            t = ytsp.tile([128, 512], BF16, tag=f"ytk{s}_{g}",
                                          name=f"ytk{s}_{g}")
                            nc.gpsimd.dma_start(
                                out=t[:],
                                in_=boutS[s][r * DPC + hh * HD:
                                             r * DPC + (hh + 1) * HD, :],
                            )
                            ytk[(s, g)] = t

                # flattened (slc, head, chunk-pair) stream, software-pipelined
                # one pair deep: scores+exp of pair i issue before sums/AV of
                # pair i-1, so the PE never waits on the ACT exp.
                items = [(s, h, p)
                         for s in range(NS) for h in range(HPC)
                         for p in range((s + 1) * 2)]
                pend = None
                upsum = {}
                for it in items:
                    s, h, p = it
                    if p == 0:
                        upsum[(s, h)] = (
                            psyt.tile([128, 512], F32, tag="pyt", name="pyt"),
                            psmb.tile([128, 512], F32, tag="psm", name="psm"),
                        )
                    psc, dlts = emit_sc(s, h, p)
                    pb = emit_exp(psc, dlts)
                    if pend is not None:
                        ps_, ph_, pp_, ppb, pdlts = pend
                        pyt, psm = upsum[(ps_, ph_)]
                        emit_con(ps_, ph_, pp_, ppb, pdlts, pyt, psm)
                        if pp_ == (ps_ + 1) * 2 - 1:   # last pair of unit
                            emit_epilogue(ps_, ph_, pyt, psm)
                            if ph_ == 3:
                                emit_gather(ps_)
                    pend = (s, h, p, pb, dlts)
                ps_, ph_, pp_, ppb, pdlts = pend
                pyt, psm = upsum[(ps_, ph_)]
                emit_con(ps_, ph_, pp_, ppb, pdlts, pyt, psm)
                emit_epilogue(ps_, ph_, pyt, psm)
                emit_gather(3)
                for s3 in range(NS):
                    load_ytk(s3)

                # phase 3, after attention: out rows 4s+j; the per-slice
                # blocks are ordered (tile_wait_until) so the PE reaches
                # block s only after its gather has landed.
                gs = [4 * r + hh for hh in range(HPC) for r in range(4)]
                for s3 in range(NS):
                    with tc.tile_wait_until(100.0 + s3):
                        for j in range(4):
                            m = 4 * s3 + j
                            po = pso.tile([128, 512], F32, tag="po", name="po") \
                                if j % 2 == 0 else \
                                psyt.tile([128, 512], F32, tag="pyt", name="po")
                            for i, g in enumerate(gs):
                                nc.tensor.matmul(
                                    po[:],
                                    ytk[(s3, g)][:, j * 128:(j + 1) * 128],
                                    wo_t[g][:],
                                    start=(i == 0), stop=(i == 15),
                                )
                            ot = ostp.tile([128, 512], F32, tag="ot", name="ot")
                            if j % 2 == 0:
                                nc.scalar.copy(ot[:], po[:])
                            else:
                                nc.vector.tensor_copy(ot[:], po[:])
                            nc.sync.dma_start(
                                out=out[m * 128:(m + 1) * 128, 0:256],
                                in_=ot[:, 0:256],
                            )
                            nc.scalar.dma_start(
                                out=out[m * 128:(m + 1) * 128, 256:512],
                                in_=ot[:, 256:512],
                            )

    nc.finalize()
    return nc


def _host_consts():
    theta = 1.0 / (BASE ** (np.arange(0, HD, 2, dtype=np.float64)[: HD // 2] / HD))
    idx = np.arange(S, dtype=np.float64)[:, None] * theta[None, :]  # [S, 64]
    cos = np.cos(idx).astype(np.float32)
    sin = np.sin(idx).astype(np.float32)
    cosE = np.repeat(cos.T, 2, axis=0)          # [128, S]
    sinE = np.repeat(sin.T, 2, axis=0)
    sinE[0::2, :] *= -1.0                        # even rows: -sin
    P = np.zeros((128, 128), np.float32)
    P[np.arange(128), np.arange(128) ^ 1] = 1.0
    btri = np.where(
        np.arange(128)[:, None] > np.arange(128)[None, :], NEG, 0.0
    ).astype(np.float32)
    ident = np.eye(128, dtype=np.float32)
    ones2 = np.ones((128, 128), np.float32)
    return {
        "cosE": cosE.astype(NPBF16),
        "sinE": sinE.astype(NPBF16),
        "pswap": P.astype(NPBF16),
        "btri": btri.astype(NPBF16),
        "ident": ident.astype(NPBF16),
        "ones2": ones2.astype(NPBF16),
    }


def kernel(x, mask, wq, wk, wv, wo):
    global LAST_EXEC_NS, LAST_TRACE
    x = np.asarray(x, dtype=np.float32)
    wq = np.asarray(wq, dtype=np.float32)
    wk = np.asarray(wk, dtype=np.float32)
    wv = np.asarray(wv, dtype=np.float32)
    wo = np.asarray(wo, dtype=np.float32)

    consts = _host_consts()
    in_maps = []
    for core in range(NCORES):
        b, j = core // 4, core % 4
        sl = slice(j * DPC, (j + 1) * DPC)
        m = {
            "xT": np.ascontiguousarray(x[b].T).astype(NPBF16),
            "wqT": np.ascontiguousarray(wq[sl, :].T).astype(NPBF16),
            "wkT": np.ascontiguousarray(wk[sl, :].T).astype(NPBF16),
            "wvT": np.ascontiguousarray(wv[sl, :].T).astype(NPBF16),
            "woT": np.ascontiguousarray(wo[sl, :].T).astype(NPBF16),
        }
        m.update(consts)
        in_maps.append(m)

    if "nc" not in _CACHE:
        _CACHE["nc"] = _build()
    nc = _CACHE["nc"]

    trace = os.environ.get("KERNEL_TRACE", "0") == "1"
    if trace:
        trace = _install_ntff_hook()
    res = run_bass_kernel_spmd(
        nc, in_maps, core_ids=list(range(NCORES)), trace=trace,
    )
    LAST_EXEC_NS = getattr(res, "exec_time_ns", None)
    LAST_TRACE = getattr(res, "instructions_and_trace", None)

    out = np.empty((B, S, D), np.float32)
    for core in range(NCORES):
        b, j = core // 4, core % 4
        out[b, :, j * DPC:(j + 1) * DPC] = np.asarray(
            res.results[core]["out"], dtype=np.float32
        )
    return out
